# revision 39
# baseline (speedup 1.0000x reference)
"""GPT-mini forward on 8 NeuronCores (Trainium2, Bass/Tile).

Sharding: core c -> (batch b=c//2, token parity par=c%2). Each core runs
the trunk for its 512 own-parity tokens; K/V need all tokens, so each
layer pair-AllGathers the LN1 output h (2MB) between the two cores of a
batch. The head computes the full 32000-vocab for the core's own tokens.
Compared to a pair-duplicated trunk this halves Q/proj/MLP/LN work.

Slot order for keys = replica-group rank order (core 2b tokens first),
so the program is SPMD-uniform; causality lives in per-core host masks.
All matmuls run as float32r (full-rate fp32 on the PE).
"""

import sys

sys.path.insert(0, "/opt/trn_rl_repo")

import numpy as np

import concourse.bass as bass
import concourse.bacc as bacc
import concourse.mybir as mybir
from concourse import tile
from concourse.bass_utils import run_bass_kernel_spmd

V, BLOCK, D, L, H, B, T = 32000, 1024, 1024, 6, 16, 4, 1024
HD = D // H          # 64
FF = 4 * D           # 4096
NCORES = 8
P = 128              # partitions
CT = D // P          # 8 c-tiles
FT = FF // P         # 32 ff-tiles
OT = T // 2          # 512 own tokens per core
KT_N = T // P        # 8 key-slot tiles of 128
VT_N = V // P        # 250 vocab tiles
GROUPS = [[0, 1], [2, 3], [4, 5], [6, 7]]

F32 = mybir.dt.float32
F32R = mybir.dt.float32r
BF16 = mybir.dt.bfloat16
AF = mybir.ActivationFunctionType
OP = mybir.AluOpType

DEBUG = False


def r(ap):
    return ap.bitcast(F32R)


def build_program():
    nc = bacc.Bacc("TRN2", target_bir_lowering=False, debug=False,
                   num_devices=NCORES)

    # ---- I/O ----  weights come packed: [.., tile, p, a*128] contiguous
    x0T = nc.declare_dram_parameter("x0T", [D, OT], F32R, isOutput=False)
    qkvw = nc.declare_dram_parameter("qkvw", [L, 3 * CT, P, D], BF16, isOutput=False)
    projw = nc.declare_dram_parameter("projw", [L, CT, P, D], BF16, isOutput=False)
    w1 = nc.declare_dram_parameter("w1", [L, FT, P, D], BF16, isOutput=False)
    w2 = nc.declare_dram_parameter("w2", [L, CT, P, FF], BF16, isOutput=False)
    headw = nc.declare_dram_parameter("headw", [VT_N, P, D], BF16, isOutput=False)
    b1 = nc.declare_dram_parameter("b1", [L, FF], F32, isOutput=False)
    b2 = nc.declare_dram_parameter("b2", [L, D], F32, isOutput=False)
    lnfg = nc.declare_dram_parameter("lnfg", [D], F32, isOutput=False)
    lnfb = nc.declare_dram_parameter("lnfb", [D], F32, isOutput=False)
    ln1g = nc.declare_dram_parameter("ln1g", [L, D], F32, isOutput=False)
    ln1b = nc.declare_dram_parameter("ln1b", [L, D], F32, isOutput=False)
    ln2g = nc.declare_dram_parameter("ln2g", [L, D], F32, isOutput=False)
    ln2b = nc.declare_dram_parameter("ln2b", [L, D], F32, isOutput=False)
    masks = nc.declare_dram_parameter("masks", [KT_N, P, OT], BF16, isOutput=False)
    onesd = nc.declare_dram_parameter("onesd", [P, 1], F32R, isOutput=False)
    identd = nc.declare_dram_parameter("identd", [P, P], F32, isOutput=False)
    out = nc.declare_dram_parameter("out", [V, OT], BF16, isOutput=True)
    if DEBUG:
        dbg_h = nc.declare_dram_parameter("dbg_h", [D, OT], BF16, isOutput=True)
        dbg_hf = nc.declare_dram_parameter("dbg_hf", [D, 2, OT], BF16, isOutput=True)
        dbg_kt = nc.declare_dram_parameter("dbg_kt", [P, T], BF16, isOutput=True)
        dbg_vx = nc.declare_dram_parameter("dbg_vx", [P, H, HD + 2], BF16, isOutput=True)
        dbg_q = nc.declare_dram_parameter("dbg_q", [P, OT], BF16, isOutput=True)
        dbg_e = nc.declare_dram_parameter("dbg_e", [P, OT], BF16, isOutput=True)
        dbg_y = nc.declare_dram_parameter("dbg_y", [P, OT], BF16, isOutput=True)
        dbg_x1 = nc.declare_dram_parameter("dbg_x1", [P, OT], F32, isOutput=True)
        dbg_x2 = nc.declare_dram_parameter("dbg_x2", [P, OT], F32, isOutput=True)

    with tile.TileContext(nc) as tc:
        with (
            nc.allow_low_precision(reason="bf16 out/masks; fp32r matmuls"),
            tc.tile_pool(name="persist", bufs=1) as pp,
            tc.tile_pool(name="hown", bufs=1) as hop,
            tc.tile_pool(name="hfull", bufs=1) as hfp,
            tc.tile_pool(name="big", bufs=1) as bigp,
            tc.tile_pool(name="qp", bufs=1) as qp,
            tc.tile_pool(name="vt", bufs=1) as vtp,
            tc.tile_pool(name="wq", bufs=3) as wqp,
            tc.tile_pool(name="w2p", bufs=2) as w2p,
            tc.tile_pool(name="ep", bufs=2) as ep,
            tc.tile_pool(name="sm", bufs=2) as smp,
            tc.tile_pool(name="ob", bufs=2) as obp,
            tc.tile_pool(name="psA", bufs=3, space="PSUM") as psA,
            tc.tile_pool(name="psB", bufs=3, space="PSUM") as psB,
            tc.tile_pool(name="dram", bufs=2, space="DRAM") as dram,
        ):
            # ---- persistent SBUF state ----
            xt = [pp.tile([P, OT], F32R, tag=f"xt{i}", name=f"xt{i}") for i in range(CT)]
            maskt = pp.tile([P, KT_N, OT], BF16, tag="masks", name="maskt")
            ones = pp.tile([P, 1], F32R, tag="ones", name="ones")
            ident = pp.tile([P, P], F32, tag="ident", name="ident")
            # ln params interleaved per c-tile: col l*2*CT + 2k = g, +1 = b
            lab = pp.tile([P, 2 * CT * L + 2 * CT], F32, tag="lnab", name="lab")
            lab2 = pp.tile([P, 2 * CT * L], F32, tag="lnab2", name="lab2")
            bb1 = pp.tile([P, FT * L], F32, tag="bb1", name="bb1")
            bb2 = pp.tile([P, CT * L], F32, tag="bb2", name="bb2")
            # heads padded to 66 cols so every per-head bf16 offset is 4B-aligned
            vx = [pp.tile([P, H, HD + 2], BF16, tag=f"vx{i}", name=f"vx{i}")
                  for i in range(KT_N)]

            nc.sync.dma_start(maskt[:], masks.ap().rearrange("a p f -> p a f"))
            nc.sync.dma_start(ones[:], onesd[:, :])
            nc.sync.dma_start(ident[:], identd[:, :])
            for i in range(CT):
                nc.sync.dma_start(xt[i][:], x0T[i * P:(i + 1) * P, :])
            for li in range(L):
                for k in range(CT):
                    o = li * 2 * CT + 2 * k
                    nc.sync.dma_start(lab[:, o:o + 1],
                                      ln1g[li, k * P:(k + 1) * P].unsqueeze(1))
                    nc.sync.dma_start(lab[:, o + 1:o + 2],
                                      ln1b[li, k * P:(k + 1) * P].unsqueeze(1))
                    nc.sync.dma_start(lab2[:, o:o + 1],
                                      ln2g[li, k * P:(k + 1) * P].unsqueeze(1))
                    nc.sync.dma_start(lab2[:, o + 1:o + 2],
                                      ln2b[li, k * P:(k + 1) * P].unsqueeze(1))
            lnf_off = L * 2 * CT
            for k in range(CT):
                nc.sync.dma_start(lab[:, lnf_off + 2 * k:lnf_off + 2 * k + 1],
                                  lnfg[k * P:(k + 1) * P].unsqueeze(1))
                nc.sync.dma_start(lab[:, lnf_off + 2 * k + 1:lnf_off + 2 * k + 2],
                                  lnfb[k * P:(k + 1) * P].unsqueeze(1))
            for li in range(L):
                for k in range(FT):
                    nc.sync.dma_start(bb1[:, li * FT + k:li * FT + k + 1],
                                      b1[li, k * P:(k + 1) * P].unsqueeze(1))
                for k in range(CT):
                    nc.sync.dma_start(bb2[:, li * CT + k:li * CT + k + 1],
                                      b2[li, k * P:(k + 1) * P].unsqueeze(1))
            # ones columns of each vx head slot, set once (V writes skip them)
            for t in range(KT_N):
                nc.vector.memset(vx[t][:, :, HD:HD + 2], 1.0)

            def layernorm(src_tiles, gb_tile, gb_off, dst_tiles):
                """dst = LN(src) over the feature axis; [P, OT] tiles."""
                s_ps = psB.tile([1, OT], F32, tag="st0", name="st0", bufs=1)
                q_ps = psB.tile([1, OT], F32, tag="st1", name="st1", bufs=1)
                for k in range(CT):
                    sq = smp.tile([P, OT], F32R, tag="scr", name="sq", bufs=3)
                    nc.vector.tensor_mul(sq[:], src_tiles[k][:], src_tiles[k][:])
                    nc.tensor.matmul(s_ps[:], r(ones[:]), r(src_tiles[k][:]),
                                     start=(k == 0), stop=(k == CT - 1))
                    nc.tensor.matmul(q_ps[:], r(ones[:]), r(sq[:]),
                                     start=(k == 0), stop=(k == CT - 1))
                mu = smp.tile([1, OT], F32, tag="st", name="mu", bufs=1)
                rstd = smp.tile([1, OT], F32, tag="st2", name="rstd", bufs=1)
                nc.vector.tensor_scalar_mul(mu[:], s_ps[:], 1.0 / D)
                nc.vector.tensor_mul(rstd[:], mu[:], mu[:])
                nc.vector.scalar_tensor_tensor(rstd[:], q_ps[:], 1.0 / D, rstd[:],
                                               OP.mult, OP.subtract)
                nc.vector.tensor_scalar_add(rstd[:], rstd[:], 1e-5)
                nc.scalar.activation(rstd[:], rstd[:], AF.Sqrt)
                nc.vector.reciprocal(rstd[:], rstd[:])
                mu_bc = ep.tile([P, OT], F32, tag="e", name="mubc")
                rs_bc = ep.tile([P, OT], F32, tag="e", name="rsbc")
                nc.gpsimd.partition_broadcast(mu_bc[:], mu[:], channels=P)
                nc.gpsimd.partition_broadcast(rs_bc[:], rstd[:], channels=P)
                for k in range(CT):
                    tmp = smp.tile([P, OT], F32, tag="scr", name="nrm", bufs=3)
                    nc.vector.tensor_sub(tmp[:], src_tiles[k][:], mu_bc[:])
                    nc.vector.tensor_mul(tmp[:], tmp[:], rs_bc[:])
                    nc.vector.tensor_scalar(
                        dst_tiles[k][:], tmp[:],
                        gb_tile[:, gb_off + 2 * k:gb_off + 2 * k + 1],
                        gb_tile[:, gb_off + 2 * k + 1:gb_off + 2 * k + 2],
                        OP.mult, OP.add)

            # =================== layers ===================
            for li in range(L):
                h = [hop.tile([P, OT], BF16, tag=f"ho{i}", name=f"ho{i}")
                     for i in range(CT)]
                layernorm(xt, lab, li * 2 * CT, h)

                # ---- exchange h within the pair (rank-major slot order) ----
                hb_in = dram.tile([D, OT], BF16, tag="hbin", name="hbin")
                hb_out = dram.tile([2, D, OT], BF16, tag="hbout", name="hbout")
                for k in range(CT):
                    nc.gpsimd.dma_start(hb_in[k * P:(k + 1) * P, :], h[k][:])
                nc.gpsimd.collective_compute(
                    "AllGather", OP.bypass, replica_groups=GROUPS,
                    ins=[hb_in.opt()], outs=[hb_out.opt()])
                hf = [hfp.tile([P, 2, OT], BF16, tag=f"hf{i}", name=f"hf{i}")
                      for i in range(CT)]
                for k in range(CT):
                    nc.sync.dma_start(
                        hf[k][:], hb_out[:, k * P:(k + 1) * P, :]
                        .rearrange("a p f -> p a f"))
                if DEBUG and li == 0:
                    for k in range(CT):
                        nc.gpsimd.dma_start(dbg_h[k * P:(k + 1) * P, :], h[k][:])
                        nc.gpsimd.dma_start(
                            dbg_hf[k * P:(k + 1) * P, :, :], hf[k][:])

                # ---- Q from own h (overlaps the collective) ----
                qy = [qp.tile([P, OT], BF16, tag=f"q{i}", name=f"qy{i}")
                      for i in range(CT)]
                for f in range(CT):
                    wt = wqp.tile([P, CT, P], BF16, tag="wq", name="wq")
                    nc.sync.dma_start(
                        wt[:], qkvw[li, f].rearrange("p (a j) -> p a j", j=P))
                    ps = psA.tile([P, OT], F32, tag="a", name="psa")
                    for k in range(CT):
                        nc.tensor.matmul(ps[:], wt[:, k, :], h[k][:],
                                         start=(k == 0), stop=(k == CT - 1))
                    nc.vector.tensor_scalar_mul(qy[f][:], ps[:], 1.0 / 8.0)

                # ---- K (tiles 8..15) and V (16..23) over all slots ----
                kt = [bigp.tile([P, T], BF16, tag=f"big{i}", name=f"kt{i}")
                      for i in range(CT)]
                for f in range(CT, 3 * CT):
                    wt = wqp.tile([P, CT, P], BF16, tag="wq", name="wq")
                    nc.sync.dma_start(
                        wt[:], qkvw[li, f].rearrange("p (a j) -> p a j", j=P))
                    if f < 2 * CT:
                        for half in range(2):
                            ps = psA.tile([P, OT], F32, tag="a", name="psa")
                            for k in range(CT):
                                nc.tensor.matmul(ps[:], wt[:, k, :],
                                                 hf[k][:, half, :],
                                                 start=(k == 0), stop=(k == CT - 1))
                            nc.vector.tensor_copy(
                                kt[f - CT][:, half * OT:(half + 1) * OT], ps[:])
                    else:
                        vtt = vtp.tile([P, T], F32, tag="vt", name="vtt")
                        for half in range(2):
                            ps = psA.tile([P, OT], F32, tag="a", name="psa")
                            for k in range(CT):
                                nc.tensor.matmul(ps[:], wt[:, k, :],
                                                 hf[k][:, half, :],
                                                 start=(k == 0), stop=(k == CT - 1))
                            nc.vector.tensor_copy(vtt[:, half * OT:(half + 1) * OT],
                                                  ps[:])
                        fb = f - 2 * CT          # feature block = heads 2fb, 2fb+1
                        h0, h1h = 2 * fb, 2 * fb + 1
                        for t in range(KT_N):
                            tp = psB.tile([P, 2, HD], F32, tag="b", name="pstr")
                            nc.tensor.transpose(tp[:, :, :].rearrange("p a f -> p (a f)"),
                                                vtt[:, t * P:(t + 1) * P],
                                                ident[:])
                            nc.vector.tensor_copy(vx[t][:, h0:h0 + 2, 0:HD], tp[:])

                if DEBUG and li == 0:
                    nc.gpsimd.dma_start(dbg_q[:, :], qy[0][:])
                    nc.gpsimd.dma_start(dbg_kt[:, :], kt[0][:])
                    nc.gpsimd.dma_start(dbg_vx[:, :, :], vx[0][:])

                # ---- attention: per head over all 8 key-slot tiles ----
                for hh in range(H):
                    ft, row = hh // 2, (hh % 2) * HD
                    av = psB.tile([HD + 2, OT], F32, tag="b", name="psav")
                    es = []
                    for ki in range(KT_N):
                        ssp = psA.tile([P, OT], F32, tag="a", name="psa")
                        nc.tensor.matmul(
                            ssp[:],
                            kt[ft][row:row + HD, ki * P:(ki + 1) * P],
                            qy[ft][row:row + HD, :],
                            start=True, stop=True)
                        e = ep.tile([P, OT], BF16, tag="es", name="e", bufs=KT_N + 1)
                        nc.scalar.activation(e[:], ssp[:], AF.Exp)
                        nc.vector.tensor_mul(e[:], e[:], maskt[:, ki, :])
                        es.append(e)
                        if DEBUG and li == 0 and hh == 0 and ki == 0:
                            nc.gpsimd.dma_start(dbg_e[:, :], e[:])
                    for ki in range(KT_N):
                        nc.tensor.matmul(
                            av[:], vx[ki][:, hh, :], es[ki][:],
                            start=(ki == 0), stop=(ki == KT_N - 1))
                    rec = smp.tile([1, OT], F32, tag="st", name="rec", bufs=1)
                    nc.vector.reciprocal(rec[:], av[HD:HD + 1, :])
                    rec_bc = ep.tile([P, OT], F32, tag="e", name="recbc")
                    nc.gpsimd.partition_broadcast(rec_bc[0:HD, :], rec[:], channels=HD)
                    nc.vector.tensor_mul(qy[ft][row:row + HD, :],
                                         av[0:HD, :], rec_bc[0:HD, :])

                if DEBUG and li == 0:
                    nc.gpsimd.dma_start(dbg_y[:, :], qy[0][:])

                # ---- proj + residual ----
                for f in range(CT):
                    wt = wqp.tile([P, CT, P], BF16, tag="wq", name="wq")
                    nc.sync.dma_start(
                        wt[:], projw[li, f].rearrange("p (a j) -> p a j", j=P))
                    ps = psA.tile([P, OT], F32, tag="a", name="psa")
                    for k in range(CT):
                        nc.tensor.matmul(ps[:], wt[:, k, :], qy[k][:],
                                         start=(k == 0), stop=(k == CT - 1))
                    nc.vector.tensor_add(xt[f][:], xt[f][:], ps[:])
                if DEBUG and li == 0:
                    nc.gpsimd.dma_start(dbg_x1[:, :], xt[0][:].bitcast(F32))

                # ---- LN2 -> h ----
                h = [hop.tile([P, OT], BF16, tag=f"ho{i}", name=f"ho{i}")
                     for i in range(CT)]
                layernorm(xt, lab2, li * 2 * CT, h)

                # ---- MLP ----
                h1t = [bigp.tile([P, 4, OT], BF16, tag=f"big{i}", name=f"h1{i}")
                       for i in range(CT)]
                for f in range(FT):
                    wt = wqp.tile([P, CT, P], BF16, tag="wq", name="wq")
                    nc.sync.dma_start(
                        wt[:], w1[li, f].rearrange("p (a j) -> p a j", j=P))
                    ps = psA.tile([P, OT], F32, tag="a", name="psa")
                    for k in range(CT):
                        nc.tensor.matmul(ps[:], wt[:, k, :], h[k][:],
                                         start=(k == 0), stop=(k == CT - 1))
                    nc.scalar.activation(
                        h1t[f // 4][:, f % 4, :], ps[:], AF.Gelu,
                        bias=bb1[:, li * FT + f:li * FT + f + 1])
                for dt_ in range(CT):
                    wt2 = w2p.tile([P, FT, P], BF16, tag="w2", name="w2t")
                    nc.sync.dma_start(
                        wt2[:], w2[li, dt_].rearrange("p (a j) -> p a j", j=P))
                    ps = psB.tile([P, OT], F32, tag="b", name="psb")
                    for fl in range(FT):
                        nc.tensor.matmul(
                            ps[:], wt2[:, fl, :],
                            h1t[fl // 4][:, fl % 4, :],
                            start=(fl == 0), stop=(fl == FT - 1))
                    nc.vector.tensor_add(xt[dt_][:], xt[dt_][:], ps[:])
                for dt_ in range(CT):
                    nc.vector.tensor_scalar_add(
                        xt[dt_][:], xt[dt_][:],
                        bb2[:, li * CT + dt_:li * CT + dt_ + 1])
                if DEBUG and li == 0:
                    nc.gpsimd.dma_start(dbg_x2[:, :], xt[0][:].bitcast(F32))

            # =================== final LN + head ===================
            h = [hop.tile([P, OT], BF16, tag=f"ho{i}", name=f"ho{i}")
                 for i in range(CT)]
            layernorm(xt, lab, lnf_off, h)
            for v in range(0, VT_N, 2):
                wt = wqp.tile([P, 2, CT, P], BF16, tag="wq", name="wq")
                nc.sync.dma_start(
                    wt[:], headw[v:v + 2].rearrange("a p (b j) -> p a b j", j=P))
                ob = obp.tile([P, 2, OT], BF16, tag="ob", name="ob")
                for u in range(2):
                    ps = psA.tile([P, OT], F32, tag="a", name="psa")
                    for k in range(CT):
                        nc.tensor.matmul(ps[:], wt[:, u, k, :], h[k][:],
                                         start=(k == 0), stop=(k == CT - 1))
                    nc.vector.tensor_copy(ob[:, u, :], ps[:])
                nc.gpsimd.dma_start(
                    out[v * P:(v + 2) * P, :].rearrange("(a p) f -> p a f", p=P),
                    ob[:])

    nc.compile()
    return nc


def _pack_k_major(w, ftiles):
    """[.., Fout, D] -> [.., ftiles, P, D] with [p, a*128+j] = w[f*128+j, a*128+p]."""
    pre = w.shape[:-2]
    wr = w.reshape(*pre, ftiles, P, CT, P)          # [.., f, j, a, p]
    wr = wr.transpose(*range(len(pre)), -4, -1, -2, -3)  # [.., f, p, a, j]
    return np.ascontiguousarray(wr.reshape(*pre, ftiles, P, D))


def prepare(idx, tok_emb, pos_emb, ln1_g, ln1_b, qkv_w, proj_w,
            ln2_g, ln2_b, mlp_w1, mlp_b1, mlp_w2, mlp_b2,
            lnf_g, lnf_b, head_w):
    idx = np.asarray(idx)
    import ml_dtypes
    f32 = lambda a: np.ascontiguousarray(np.asarray(a), dtype=np.float32)
    bf = lambda a: np.ascontiguousarray(a.astype(ml_dtypes.bfloat16))

    tok_emb, pos_emb = f32(tok_emb), f32(pos_emb)
    qkvw_p = bf(_pack_k_major(f32(qkv_w), 3 * CT))   # [L, 24, P, D]
    projw_p = bf(_pack_k_major(f32(proj_w), CT))     # [L, 8, P, D]
    w1_p = bf(_pack_k_major(f32(mlp_w1), FT))        # [L, 32, P, D]
    # w2: [L, D, FF] -> per d-tile [P, FF]: [p, a*128+j] = w2[dt*128+j, a*128+p]
    w2r = f32(mlp_w2).reshape(L, CT, P, FT, P)       # [L, dt, j, a, p]
    w2_p = bf(w2r.transpose(0, 1, 4, 3, 2).reshape(L, CT, P, FF))
    headw_p = bf(_pack_k_major(f32(head_w), VT_N))   # [250, P, D]

    x0 = tok_emb[idx] + pos_emb[0][None, :, :]       # [B, T, D]

    onesv = np.ones((P, 1), np.float32)
    ident = np.eye(P, dtype=np.float32)
    common = dict(qkvw=qkvw_p, projw=projw_p, w1=w1_p, w2=w2_p, headw=headw_p,
                  ln1g=f32(ln1_g), ln1b=f32(ln1_b), ln2g=f32(ln2_g),
                  ln2b=f32(ln2_b), b1=f32(mlp_b1), b2=f32(mlp_b2),
                  lnfg=f32(lnf_g), lnfb=f32(lnf_b),
                  onesd=onesv, identd=ident)

    slot_global = np.empty(T, np.int64)              # key slot -> global token
    slot_global[:OT] = 2 * np.arange(OT)             # rank 0 = even tokens
    slot_global[OT:] = 2 * np.arange(OT) + 1         # rank 1 = odd tokens

    in_maps = []
    for c in range(NCORES):
        b, par = c // 2, c % 2
        own = np.arange(par, T, 2)                   # global own-token ids
        x0T = np.ascontiguousarray(x0[b][own].T)     # [D, 512]
        mask = (slot_global[:, None] <= own[None, :])   # [1024 slots, 512 q]
        mask = mask.reshape(KT_N, P, OT).astype(ml_dtypes.bfloat16)
        m = dict(common)
        m["x0T"] = x0T
        m["masks"] = np.ascontiguousarray(mask)
        in_maps.append(m)
    return in_maps


_NC_CACHE = None


def get_program():
    global _NC_CACHE
    if _NC_CACHE is None:
        _NC_CACHE = build_program()
    return _NC_CACHE


def assemble(results):
    logits = np.empty((B, T, V), np.float32)
    for c in range(NCORES):
        b, par = c // 2, c % 2
        o = np.asarray(results[c]["out"], dtype=np.float32)   # [V, 512]
        logits[b, par::2, :] = o.T
    return logits


def kernel(idx, tok_emb, pos_emb, ln1_g, ln1_b, qkv_w, proj_w,
           ln2_g, ln2_b, mlp_w1, mlp_b1, mlp_w2, mlp_b2,
           lnf_g, lnf_b, head_w, _trace=False):
    in_maps = prepare(idx, tok_emb, pos_emb, ln1_g, ln1_b, qkv_w, proj_w,
                      ln2_g, ln2_b, mlp_w1, mlp_b1, mlp_w2, mlp_b2,
                      lnf_g, lnf_b, head_w)
    nc = get_program()
    res = run_bass_kernel_spmd(nc, in_maps, list(range(NCORES)))
    if getattr(res, "exec_time_ns", None):
        print(f"HW exec time: {res.exec_time_ns} ns")
    return assemble(res.results)


# revision 40
# speedup vs baseline: 1.1121x; 1.1121x over previous
"""GPT-mini forward on 8 NeuronCores (Trainium2, Bass/Tile).

Sharding: core c -> (batch b=c//2, token parity par=c%2). Each core runs
the trunk for its 512 own-parity tokens; K/V need all tokens, so each
layer pair-AllGathers the LN1 output h (2MB) between the two cores of a
batch. The head computes the full 32000-vocab for the core's own tokens.
Compared to a pair-duplicated trunk this halves Q/proj/MLP/LN work.

Slot order for keys = replica-group rank order (core 2b tokens first),
so the program is SPMD-uniform; causality lives in per-core host masks.
All matmuls run as float32r (full-rate fp32 on the PE).
"""

import sys

sys.path.insert(0, "/opt/trn_rl_repo")

import numpy as np

import concourse.bass as bass
import concourse.bacc as bacc
import concourse.mybir as mybir
from concourse import tile
from concourse.bass_utils import run_bass_kernel_spmd

V, BLOCK, D, L, H, B, T = 32000, 1024, 1024, 6, 16, 4, 1024
HD = D // H          # 64
FF = 4 * D           # 4096
NCORES = 8
P = 128              # partitions
CT = D // P          # 8 c-tiles
FT = FF // P         # 32 ff-tiles
OT = T // 2          # 512 own tokens per core
KT_N = T // P        # 8 key-slot tiles of 128
VT_N = V // P        # 250 vocab tiles
GROUPS = [[0, 1], [2, 3], [4, 5], [6, 7]]

F32 = mybir.dt.float32
F32R = mybir.dt.float32r
BF16 = mybir.dt.bfloat16
AF = mybir.ActivationFunctionType
OP = mybir.AluOpType

DEBUG = False


def r(ap):
    return ap.bitcast(F32R)


def build_program():
    nc = bacc.Bacc("TRN2", target_bir_lowering=False, debug=False,
                   num_devices=NCORES)

    # ---- I/O ----  weights come packed: [.., tile, p, a*128] contiguous
    x0T = nc.declare_dram_parameter("x0T", [D, OT], F32R, isOutput=False)
    qkvw = nc.declare_dram_parameter("qkvw", [L, 3 * CT, P, D], BF16, isOutput=False)
    projw = nc.declare_dram_parameter("projw", [L, CT, P, D], BF16, isOutput=False)
    w1 = nc.declare_dram_parameter("w1", [L, FT, P, D], BF16, isOutput=False)
    w2 = nc.declare_dram_parameter("w2", [L, CT, P, FF], BF16, isOutput=False)
    headw = nc.declare_dram_parameter("headw", [VT_N, P, D], BF16, isOutput=False)
    b1 = nc.declare_dram_parameter("b1", [L, FF], F32, isOutput=False)
    b2 = nc.declare_dram_parameter("b2", [L, D], F32, isOutput=False)
    lnfg = nc.declare_dram_parameter("lnfg", [D], F32, isOutput=False)
    lnfb = nc.declare_dram_parameter("lnfb", [D], F32, isOutput=False)
    ln1g = nc.declare_dram_parameter("ln1g", [L, D], F32, isOutput=False)
    ln1b = nc.declare_dram_parameter("ln1b", [L, D], F32, isOutput=False)
    ln2g = nc.declare_dram_parameter("ln2g", [L, D], F32, isOutput=False)
    ln2b = nc.declare_dram_parameter("ln2b", [L, D], F32, isOutput=False)
    masks = nc.declare_dram_parameter("masks", [KT_N, P, OT], BF16, isOutput=False)
    onesd = nc.declare_dram_parameter("onesd", [P, 1], F32R, isOutput=False)
    identd = nc.declare_dram_parameter("identd", [P, P], F32, isOutput=False)
    out = nc.declare_dram_parameter("out", [V, OT], BF16, isOutput=True)
    if DEBUG:
        dbg_h = nc.declare_dram_parameter("dbg_h", [D, OT], BF16, isOutput=True)
        dbg_hf = nc.declare_dram_parameter("dbg_hf", [D, 2, OT], BF16, isOutput=True)
        dbg_kt = nc.declare_dram_parameter("dbg_kt", [P, T], BF16, isOutput=True)
        dbg_vx = nc.declare_dram_parameter("dbg_vx", [P, H, HD + 2], BF16, isOutput=True)
        dbg_q = nc.declare_dram_parameter("dbg_q", [P, OT], BF16, isOutput=True)
        dbg_e = nc.declare_dram_parameter("dbg_e", [P, OT], BF16, isOutput=True)
        dbg_y = nc.declare_dram_parameter("dbg_y", [P, OT], BF16, isOutput=True)
        dbg_x1 = nc.declare_dram_parameter("dbg_x1", [P, OT], F32, isOutput=True)
        dbg_x2 = nc.declare_dram_parameter("dbg_x2", [P, OT], F32, isOutput=True)

    with tile.TileContext(nc) as tc:
        with (
            nc.allow_low_precision(reason="bf16 out/masks; fp32r matmuls"),
            tc.tile_pool(name="persist", bufs=1) as pp,
            tc.tile_pool(name="hown", bufs=1) as hop,
            tc.tile_pool(name="hfull", bufs=1) as hfp,
            tc.tile_pool(name="big", bufs=1) as bigp,
            tc.tile_pool(name="qp", bufs=1) as qp,
            tc.tile_pool(name="vt", bufs=1) as vtp,
            tc.tile_pool(name="wq", bufs=3) as wqp,
            tc.tile_pool(name="w2p", bufs=2) as w2p,
            tc.tile_pool(name="ep", bufs=2) as ep,
            tc.tile_pool(name="sm", bufs=2) as smp,
            tc.tile_pool(name="ob", bufs=2) as obp,
            tc.tile_pool(name="psA", bufs=3, space="PSUM") as psA,
            tc.tile_pool(name="psB", bufs=3, space="PSUM") as psB,
            tc.tile_pool(name="dram", bufs=2, space="DRAM") as dram,
        ):
            # ---- persistent SBUF state ----
            xt = [pp.tile([P, OT], F32R, tag=f"xt{i}", name=f"xt{i}") for i in range(CT)]
            maskt = pp.tile([P, KT_N, OT], BF16, tag="masks", name="maskt")
            ones = pp.tile([P, 1], F32R, tag="ones", name="ones")
            ident = pp.tile([P, P], F32, tag="ident", name="ident")
            # ln params interleaved per c-tile: col l*2*CT + 2k = g, +1 = b
            lab = pp.tile([P, 2 * CT * L + 2 * CT], F32, tag="lnab", name="lab")
            lab2 = pp.tile([P, 2 * CT * L], F32, tag="lnab2", name="lab2")
            bb1 = pp.tile([P, FT * L], F32, tag="bb1", name="bb1")
            bb2 = pp.tile([P, CT * L], F32, tag="bb2", name="bb2")
            # heads padded to 66 cols so every per-head bf16 offset is 4B-aligned
            vx = [pp.tile([P, H, HD + 2], BF16, tag=f"vx{i}", name=f"vx{i}")
                  for i in range(KT_N)]
            vxo = [pp.tile([P, H, HD + 2], BF16, tag=f"vxo{i}", name=f"vxo{i}")
                   for i in range(KT_N // 2)]

            nc.sync.dma_start(maskt[:], masks.ap().rearrange("a p f -> p a f"))
            nc.sync.dma_start(ones[:], onesd[:, :])
            nc.sync.dma_start(ident[:], identd[:, :])
            for i in range(CT):
                nc.sync.dma_start(xt[i][:], x0T[i * P:(i + 1) * P, :])
            for li in range(L):
                for k in range(CT):
                    o = li * 2 * CT + 2 * k
                    nc.sync.dma_start(lab[:, o:o + 1],
                                      ln1g[li, k * P:(k + 1) * P].unsqueeze(1))
                    nc.sync.dma_start(lab[:, o + 1:o + 2],
                                      ln1b[li, k * P:(k + 1) * P].unsqueeze(1))
                    nc.sync.dma_start(lab2[:, o:o + 1],
                                      ln2g[li, k * P:(k + 1) * P].unsqueeze(1))
                    nc.sync.dma_start(lab2[:, o + 1:o + 2],
                                      ln2b[li, k * P:(k + 1) * P].unsqueeze(1))
            lnf_off = L * 2 * CT
            for k in range(CT):
                nc.sync.dma_start(lab[:, lnf_off + 2 * k:lnf_off + 2 * k + 1],
                                  lnfg[k * P:(k + 1) * P].unsqueeze(1))
                nc.sync.dma_start(lab[:, lnf_off + 2 * k + 1:lnf_off + 2 * k + 2],
                                  lnfb[k * P:(k + 1) * P].unsqueeze(1))
            for li in range(L):
                for k in range(FT):
                    nc.sync.dma_start(bb1[:, li * FT + k:li * FT + k + 1],
                                      b1[li, k * P:(k + 1) * P].unsqueeze(1))
                for k in range(CT):
                    nc.sync.dma_start(bb2[:, li * CT + k:li * CT + k + 1],
                                      b2[li, k * P:(k + 1) * P].unsqueeze(1))
            # ones columns of each own-V slot tile, set once (V writes skip
            # them; vx inherits them through the gather)
            for t in range(KT_N // 2):
                nc.vector.memset(vxo[t][:, :, HD:HD + 2], 1.0)

            KSZ = P * OT                 # elements per K f-tile in the bounce
            VSZ = P * H * (HD + 2)       # elements per V slot tile
            KREG = CT * KSZ              # K region size

            def layernorm(src_tiles, gb_tile, gb_off, dst_tiles):
                """dst = LN(src) over the feature axis; [P, OT] tiles."""
                s_ps = psB.tile([1, OT], F32, tag="st0", name="st0", bufs=1)
                q_ps = psB.tile([1, OT], F32, tag="st1", name="st1", bufs=1)
                for k in range(CT):
                    sq = smp.tile([P, OT], F32R, tag="scr", name="sq", bufs=3)
                    nc.vector.tensor_mul(sq[:], src_tiles[k][:], src_tiles[k][:])
                    nc.tensor.matmul(s_ps[:], r(ones[:]), r(src_tiles[k][:]),
                                     start=(k == 0), stop=(k == CT - 1))
                    nc.tensor.matmul(q_ps[:], r(ones[:]), r(sq[:]),
                                     start=(k == 0), stop=(k == CT - 1))
                mu = smp.tile([1, OT], F32, tag="st", name="mu", bufs=1)
                rstd = smp.tile([1, OT], F32, tag="st2", name="rstd", bufs=1)
                nc.vector.tensor_scalar_mul(mu[:], s_ps[:], 1.0 / D)
                nc.vector.tensor_mul(rstd[:], mu[:], mu[:])
                nc.vector.scalar_tensor_tensor(rstd[:], q_ps[:], 1.0 / D, rstd[:],
                                               OP.mult, OP.subtract)
                nc.vector.tensor_scalar_add(rstd[:], rstd[:], 1e-5)
                nc.scalar.activation(rstd[:], rstd[:], AF.Sqrt)
                nc.vector.reciprocal(rstd[:], rstd[:])
                mu_bc = ep.tile([P, OT], F32, tag="e", name="mubc")
                rs_bc = ep.tile([P, OT], F32, tag="e", name="rsbc")
                nc.gpsimd.partition_broadcast(mu_bc[:], mu[:], channels=P)
                nc.gpsimd.partition_broadcast(rs_bc[:], rstd[:], channels=P)
                for k in range(CT):
                    tmp = smp.tile([P, OT], F32, tag="scr", name="nrm", bufs=3)
                    nc.vector.tensor_sub(tmp[:], src_tiles[k][:], mu_bc[:])
                    nc.vector.tensor_mul(tmp[:], tmp[:], rs_bc[:])
                    nc.vector.tensor_scalar(
                        dst_tiles[k][:], tmp[:],
                        gb_tile[:, gb_off + 2 * k:gb_off + 2 * k + 1],
                        gb_tile[:, gb_off + 2 * k + 1:gb_off + 2 * k + 2],
                        OP.mult, OP.add)

            # =================== layers ===================
            for li in range(L):
                h = [hop.tile([P, OT], BF16, tag=f"ho{i}", name=f"ho{i}")
                     for i in range(CT)]
                layernorm(xt, lab, li * 2 * CT, h)

                # ---- K,V for own tokens only, then pair-AllGather K/V ----
                kvb_in = dram.tile([KREG + (KT_N // 2) * VSZ], BF16,
                                   tag="kvbin", name="kvbin")
                kvb_out = dram.tile([2, KREG + (KT_N // 2) * VSZ], BF16,
                                    tag="kvbout", name="kvbout")
                ko = [hfp.tile([P, OT], BF16, tag=f"hf{i}", name=f"ko{i}")
                      for i in range(CT)]
                for f in range(CT, 3 * CT):
                    wt = wqp.tile([P, CT, P], BF16, tag="wq", name="wq")
                    nc.sync.dma_start(
                        wt[:], qkvw[li, f].rearrange("p (a j) -> p a j", j=P))
                    ps = psA.tile([P, OT], F32, tag="a", name="psa")
                    for k in range(CT):
                        nc.tensor.matmul(ps[:], wt[:, k, :], h[k][:],
                                         start=(k == 0), stop=(k == CT - 1))
                    if f < 2 * CT:
                        fk = f - CT
                        nc.vector.tensor_copy(ko[fk][:], ps[:])
                        nc.gpsimd.dma_start(
                            kvb_in[fk * KSZ:(fk + 1) * KSZ]
                            .rearrange("(p t) -> p t", p=P), ko[fk][:])
                    else:
                        vtt = vtp.tile([P, OT], F32, tag="vt", name="vtt")
                        nc.vector.tensor_copy(vtt[:], ps[:])
                        fb = f - 2 * CT          # feature block = heads 2fb, 2fb+1
                        h0 = 2 * fb
                        for t in range(KT_N // 2):
                            tp = psB.tile([P, 2, HD], F32, tag="b", name="pstr")
                            nc.tensor.transpose(
                                tp[:, :, :].rearrange("p a f -> p (a f)"),
                                vtt[:, t * P:(t + 1) * P], ident[:])
                            nc.vector.tensor_copy(vxo[t][:, h0:h0 + 2, 0:HD], tp[:])
                for t in range(KT_N // 2):
                    nc.gpsimd.dma_start(
                        kvb_in[KREG + t * VSZ:KREG + (t + 1) * VSZ]
                        .rearrange("(p h c) -> p h c", p=P, h=H), vxo[t][:])
                nc.gpsimd.collective_compute(
                    "AllGather", OP.bypass, replica_groups=GROUPS,
                    ins=[kvb_in.opt()], outs=[kvb_out.opt()])

                # ---- Q from own h (overlaps the collective) ----
                qy = [qp.tile([P, OT], BF16, tag=f"q{i}", name=f"qy{i}")
                      for i in range(CT)]
                for f in range(CT):
                    wt = wqp.tile([P, CT, P], BF16, tag="wq", name="wq")
                    nc.sync.dma_start(
                        wt[:], qkvw[li, f].rearrange("p (a j) -> p a j", j=P))
                    ps = psA.tile([P, OT], F32, tag="a", name="psa")
                    for k in range(CT):
                        nc.tensor.matmul(ps[:], wt[:, k, :], h[k][:],
                                         start=(k == 0), stop=(k == CT - 1))
                    nc.vector.tensor_scalar_mul(qy[f][:], ps[:], 1.0 / 8.0)

                # ---- import gathered K/V (rank-major slot order) ----
                kt = [bigp.tile([P, T], BF16, tag=f"big{i}", name=f"kt{i}")
                      for i in range(CT)]
                for half in range(2):
                    for k in range(CT):
                        nc.sync.dma_start(
                            kt[k][:, half * OT:(half + 1) * OT],
                            kvb_out[half, k * KSZ:(k + 1) * KSZ]
                            .rearrange("(p t) -> p t", p=P))
                    for t in range(KT_N // 2):
                        nc.sync.dma_start(
                            vx[half * (KT_N // 2) + t][:],
                            kvb_out[half, KREG + t * VSZ:KREG + (t + 1) * VSZ]
                            .rearrange("(p h c) -> p h c", p=P, h=H))

                if DEBUG and li == 0:
                    nc.gpsimd.dma_start(dbg_q[:, :], qy[0][:])
                    nc.gpsimd.dma_start(dbg_kt[:, :], kt[0][:])
                    nc.gpsimd.dma_start(dbg_vx[:, :, :], vx[0][:])

                # ---- attention: per head over all 8 key-slot tiles ----
                for hh in range(H):
                    ft, row = hh // 2, (hh % 2) * HD
                    av = psB.tile([HD + 2, OT], F32, tag="b", name="psav")
                    es = []
                    for ki in range(KT_N):
                        ssp = psA.tile([P, OT], F32, tag="a", name="psa")
                        nc.tensor.matmul(
                            ssp[:],
                            kt[ft][row:row + HD, ki * P:(ki + 1) * P],
                            qy[ft][row:row + HD, :],
                            start=True, stop=True)
                        e = ep.tile([P, OT], BF16, tag="es", name="e", bufs=KT_N + 1)
                        nc.scalar.activation(e[:], ssp[:], AF.Exp)
                        nc.vector.tensor_mul(e[:], e[:], maskt[:, ki, :])
                        es.append(e)
                        if DEBUG and li == 0 and hh == 0 and ki == 0:
                            nc.gpsimd.dma_start(dbg_e[:, :], e[:])
                    for ki in range(KT_N):
                        nc.tensor.matmul(
                            av[:], vx[ki][:, hh, :], es[ki][:],
                            start=(ki == 0), stop=(ki == KT_N - 1))
                    rec = smp.tile([1, OT], F32, tag="st", name="rec", bufs=1)
                    nc.vector.reciprocal(rec[:], av[HD:HD + 1, :])
                    rec_bc = ep.tile([P, OT], F32, tag="e", name="recbc")
                    nc.gpsimd.partition_broadcast(rec_bc[0:HD, :], rec[:], channels=HD)
                    nc.vector.tensor_mul(qy[ft][row:row + HD, :],
                                         av[0:HD, :], rec_bc[0:HD, :])

                if DEBUG and li == 0:
                    nc.gpsimd.dma_start(dbg_y[:, :], qy[0][:])

                # ---- proj + residual ----
                for f in range(CT):
                    wt = wqp.tile([P, CT, P], BF16, tag="wq", name="wq")
                    nc.sync.dma_start(
                        wt[:], projw[li, f].rearrange("p (a j) -> p a j", j=P))
                    ps = psA.tile([P, OT], F32, tag="a", name="psa")
                    for k in range(CT):
                        nc.tensor.matmul(ps[:], wt[:, k, :], qy[k][:],
                                         start=(k == 0), stop=(k == CT - 1))
                    nc.vector.tensor_add(xt[f][:], xt[f][:], ps[:])
                if DEBUG and li == 0:
                    nc.gpsimd.dma_start(dbg_x1[:, :], xt[0][:].bitcast(F32))

                # ---- LN2 -> h ----
                h = [hop.tile([P, OT], BF16, tag=f"ho{i}", name=f"ho{i}")
                     for i in range(CT)]
                layernorm(xt, lab2, li * 2 * CT, h)

                # ---- MLP ----
                h1t = [bigp.tile([P, 4, OT], BF16, tag=f"big{i}", name=f"h1{i}")
                       for i in range(CT)]
                for f in range(FT):
                    wt = wqp.tile([P, CT, P], BF16, tag="wq", name="wq")
                    nc.sync.dma_start(
                        wt[:], w1[li, f].rearrange("p (a j) -> p a j", j=P))
                    ps = psA.tile([P, OT], F32, tag="a", name="psa")
                    for k in range(CT):
                        nc.tensor.matmul(ps[:], wt[:, k, :], h[k][:],
                                         start=(k == 0), stop=(k == CT - 1))
                    nc.scalar.activation(
                        h1t[f // 4][:, f % 4, :], ps[:], AF.Gelu,
                        bias=bb1[:, li * FT + f:li * FT + f + 1])
                for dt_ in range(CT):
                    wt2 = w2p.tile([P, FT, P], BF16, tag="w2", name="w2t")
                    nc.sync.dma_start(
                        wt2[:], w2[li, dt_].rearrange("p (a j) -> p a j", j=P))
                    ps = psB.tile([P, OT], F32, tag="b", name="psb")
                    for fl in range(FT):
                        nc.tensor.matmul(
                            ps[:], wt2[:, fl, :],
                            h1t[fl // 4][:, fl % 4, :],
                            start=(fl == 0), stop=(fl == FT - 1))
                    nc.vector.tensor_add(xt[dt_][:], xt[dt_][:], ps[:])
                for dt_ in range(CT):
                    nc.vector.tensor_scalar_add(
                        xt[dt_][:], xt[dt_][:],
                        bb2[:, li * CT + dt_:li * CT + dt_ + 1])
                if DEBUG and li == 0:
                    nc.gpsimd.dma_start(dbg_x2[:, :], xt[0][:].bitcast(F32))

            # =================== final LN + head ===================
            h = [hop.tile([P, OT], BF16, tag=f"ho{i}", name=f"ho{i}")
                 for i in range(CT)]
            layernorm(xt, lab, lnf_off, h)
            for v in range(0, VT_N, 2):
                wt = wqp.tile([P, 2, CT, P], BF16, tag="wq", name="wq")
                nc.sync.dma_start(
                    wt[:], headw[v:v + 2].rearrange("a p (b j) -> p a b j", j=P))
                ob = obp.tile([P, 2, OT], BF16, tag="ob", name="ob")
                for u in range(2):
                    ps = psA.tile([P, OT], F32, tag="a", name="psa")
                    for k in range(CT):
                        nc.tensor.matmul(ps[:], wt[:, u, k, :], h[k][:],
                                         start=(k == 0), stop=(k == CT - 1))
                    nc.vector.tensor_copy(ob[:, u, :], ps[:])
                nc.gpsimd.dma_start(
                    out[v * P:(v + 2) * P, :].rearrange("(a p) f -> p a f", p=P),
                    ob[:])

    nc.compile()
    return nc


def _pack_k_major(w, ftiles):
    """[.., Fout, D] -> [.., ftiles, P, D] with [p, a*128+j] = w[f*128+j, a*128+p]."""
    pre = w.shape[:-2]
    wr = w.reshape(*pre, ftiles, P, CT, P)          # [.., f, j, a, p]
    wr = wr.transpose(*range(len(pre)), -4, -1, -2, -3)  # [.., f, p, a, j]
    return np.ascontiguousarray(wr.reshape(*pre, ftiles, P, D))


def prepare(idx, tok_emb, pos_emb, ln1_g, ln1_b, qkv_w, proj_w,
            ln2_g, ln2_b, mlp_w1, mlp_b1, mlp_w2, mlp_b2,
            lnf_g, lnf_b, head_w):
    idx = np.asarray(idx)
    import ml_dtypes
    f32 = lambda a: np.ascontiguousarray(np.asarray(a), dtype=np.float32)
    bf = lambda a: np.ascontiguousarray(a.astype(ml_dtypes.bfloat16))

    tok_emb, pos_emb = f32(tok_emb), f32(pos_emb)
    qkvw_p = bf(_pack_k_major(f32(qkv_w), 3 * CT))   # [L, 24, P, D]
    projw_p = bf(_pack_k_major(f32(proj_w), CT))     # [L, 8, P, D]
    w1_p = bf(_pack_k_major(f32(mlp_w1), FT))        # [L, 32, P, D]
    # w2: [L, D, FF] -> per d-tile [P, FF]: [p, a*128+j] = w2[dt*128+j, a*128+p]
    w2r = f32(mlp_w2).reshape(L, CT, P, FT, P)       # [L, dt, j, a, p]
    w2_p = bf(w2r.transpose(0, 1, 4, 3, 2).reshape(L, CT, P, FF))
    headw_p = bf(_pack_k_major(f32(head_w), VT_N))   # [250, P, D]

    x0 = tok_emb[idx] + pos_emb[0][None, :, :]       # [B, T, D]

    onesv = np.ones((P, 1), np.float32)
    ident = np.eye(P, dtype=np.float32)
    common = dict(qkvw=qkvw_p, projw=projw_p, w1=w1_p, w2=w2_p, headw=headw_p,
                  ln1g=f32(ln1_g), ln1b=f32(ln1_b), ln2g=f32(ln2_g),
                  ln2b=f32(ln2_b), b1=f32(mlp_b1), b2=f32(mlp_b2),
                  lnfg=f32(lnf_g), lnfb=f32(lnf_b),
                  onesd=onesv, identd=ident)

    slot_global = np.empty(T, np.int64)              # key slot -> global token
    slot_global[:OT] = 2 * np.arange(OT)             # rank 0 = even tokens
    slot_global[OT:] = 2 * np.arange(OT) + 1         # rank 1 = odd tokens

    in_maps = []
    for c in range(NCORES):
        b, par = c // 2, c % 2
        own = np.arange(par, T, 2)                   # global own-token ids
        x0T = np.ascontiguousarray(x0[b][own].T)     # [D, 512]
        mask = (slot_global[:, None] <= own[None, :])   # [1024 slots, 512 q]
        mask = mask.reshape(KT_N, P, OT).astype(ml_dtypes.bfloat16)
        m = dict(common)
        m["x0T"] = x0T
        m["masks"] = np.ascontiguousarray(mask)
        in_maps.append(m)
    return in_maps


_NC_CACHE = None


def get_program():
    global _NC_CACHE
    if _NC_CACHE is None:
        _NC_CACHE = build_program()
    return _NC_CACHE


def assemble(results):
    logits = np.empty((B, T, V), np.float32)
    for c in range(NCORES):
        b, par = c // 2, c % 2
        o = np.asarray(results[c]["out"], dtype=np.float32)   # [V, 512]
        logits[b, par::2, :] = o.T
    return logits


def kernel(idx, tok_emb, pos_emb, ln1_g, ln1_b, qkv_w, proj_w,
           ln2_g, ln2_b, mlp_w1, mlp_b1, mlp_w2, mlp_b2,
           lnf_g, lnf_b, head_w, _trace=False):
    in_maps = prepare(idx, tok_emb, pos_emb, ln1_g, ln1_b, qkv_w, proj_w,
                      ln2_g, ln2_b, mlp_w1, mlp_b1, mlp_w2, mlp_b2,
                      lnf_g, lnf_b, head_w)
    nc = get_program()
    res = run_bass_kernel_spmd(nc, in_maps, list(range(NCORES)))
    if getattr(res, "exec_time_ns", None):
        print(f"HW exec time: {res.exec_time_ns} ns")
    return assemble(res.results)
